# revision 6
# baseline (speedup 1.0000x reference)
"""Trainium2 Bass kernel for nn_Decoder_10110353014984.

Computation (see reference): hard-reset LIF over T=4 steps followed by a
linear head:
    v' = v + (x_t - v)/2 ; spike = (v' >= 1) ; v = (1-spike) * v'
    y  = einsum('tbnd,cd->tbnc', spikes, W) + b

The LIF replicates the reference's exact fp32 rounding order:
    h = (x*1 - v) ; v' = h*0.5 + v ; spike = v' >= 1 ; v = (v' < 1) * v'
(x*1 and h*0.5 are exact, so the rounding sequence matches v + (x-v)/2).
Exactness matters: a single spike flip changes one output row by a full
W column (~0.2 abs) and would blow the error budget.

Sharding: data-parallel over batch B=64 -> 8 per NeuronCore. The host
pre-transposes each shard to xT[T, D, S] (d-major) so LIF spike tiles are
directly the matmul stationary operand; W is shipped as the exact flat
SBUF image wf[128, 4000] (bf16, ci-major halves) so it loads with one
8KB-per-partition descriptor DMA.

Numerics: spikes {0,1} exact in bf16; W cast to bf16 on host; y stored
fp16 on device and upcast on host (combined rel err ~2e-3, well under
the 2e-2 gate).

DMA plan (the run is co-limited by the single HWDGE ring in the
baseline): x bulk loads ride the GpSimd *software* DGE queue while W,
the startup x range, and all y stores ride the Sync HWDGE ring - two
descriptor-generation feeds in parallel. y goes out as [T, 6, P, 2000]
(4KB per-partition descriptors) plus a packed 32-row tail tensor, and
the host untangles the layout. x(t+1) is prefetched at the start of
timestep t.

Engine placement: LIF reset/charge on DVE (irreducible two-tensor fp32
chains); thresholds split DVE/Scalar by sample range to balance both
engines just under the PE's ~88us matmul stream; PSUM->SBUF(fp16)
copies on Scalar grouped 2 sample-chunks x 2 C-halves per instruction.
The four 32-row tails (S = 12*128 + 32) are packed across t into one
full 128-row matmul chunk at the end.
"""

import sys
import types

sys.path.insert(0, "/opt/trn_rl_repo")

import numpy as np
import ml_dtypes

import concourse.bass as bass
import concourse.mybir as mybir
import concourse.tile as tile
from concourse.vector_clock import ScopedClock
import bass_rust as _br

T, B, N, D, C = 4, 64, 196, 512, 1000
NCORES = 8
BL = B // NCORES          # 8 batches per core
S = BL * N                # 1568 samples per timestep per core
P = 128                   # partition width
DCH = D // P              # 4 contraction tiles
CHALF = [(0, 500), (500, 500)]  # C split across two PSUM banks
# sample chunks paired per PSUM group; the 32-row tail is packed across
# the 4 timesteps into one 128-row chunk at the end
GROUPS = [(0, 1), (2, 3), (4, 5), (6, 7), (8, 9), (10, 11)]
NG = len(GROUPS)
SMAIN = 12 * P            # 1536 samples in the paired groups
STAIL = S - SMAIN         # 32 tail samples per timestep

# LIF / threshold sample ranges (r3 carries the 32-sample tail)
RANGES = [(0, 256), (256, 512), (768, 512), (1280, 288)]
# threshold engine per range: 'V' = DVE is_ge (1 pass), 'S' = Scalar
# Sign+Relu (2 passes).  Split tuned so DVE ~ Scalar ~ just under PE.
THRESH_ENG = ["V", "S", "S", "V"]
# t=0 charge (0.5*x, single-tensor) engine per range; all DVE -- the
# Scalar engine is the tighter of the two overall
CHARGE0_ENG = ["V", "V", "V", "V"]

F32 = mybir.dt.float32
F16 = mybir.dt.float16
BF16 = mybir.dt.bfloat16
ALU = mybir.AluOpType
ACTF = mybir.ActivationFunctionType


def _patch_tile_drain():
    """This walrus build allows at most one sync wait per TPB_CTRL (Drain)
    instruction; Tile's tail drain carries one wait per active processor.
    Split it into a chain of single-wait drains (same-engine program order
    makes the conjunction equivalent)."""
    if getattr(tile.TileContext, "_drain_split_patch", False):
        return

    def _drain_and_barrier(self, tick_clock, wait_clock):
        drain_inst = self.nc.sync.drain()
        wait_clock.add_sem_waits(
            drain_inst.ins, ScopedClock({None: tick_clock.global_clock})
        )
        waits = (
            list(drain_inst.ins.sync_info.on_wait)
            if drain_inst.ins.has_wait()
            else []
        )
        if len(waits) > 1:
            drain_inst.ins.sync_info.on_wait = waits[:1]
            for i in range(1, len(waits)):
                d2 = self.nc.sync.drain()
                d2.ins.sync_info = _br.SyncInfo(on_wait=waits[i : i + 1], on_update=[])
        self.nc.all_engine_barrier()
        assert self.sems is not None
        popped = self.nc._tile_sem_poison_stack.pop()
        assert popped is self._sem_poison
        self.nc.clear_and_free_semaphores(list(self.sems.allocated().values()))
        self.nc.all_engine_barrier()

    tile.TileContext._drain_and_barrier = _drain_and_barrier

    # Same limit applies to every instruction class (Matmult, DMACopy, ...).
    # Before committing the scheduled instruction stream, shed all but one
    # wait per instruction onto standalone same-engine InstEventSemaphore
    # carriers placed immediately before it (engine program order preserves
    # the conjunction).
    _orig_lower = tile.TileContext._lower_ordered_insts

    def _split_lower(self, ordered):
        for bb_name, insts in ordered.items():
            new = []
            for inst in insts:
                si = inst.sync_info
                if si is not None and len(si.on_wait) > 1:
                    waits = list(si.on_wait)
                    for w in waits[:-1]:
                        ev = mybir.InstEventSemaphore(
                            name=self.nc.get_next_instruction_name(), ins=[], outs=[]
                        )
                        ev.engine = inst.engine
                        ev.sync_info = _br.SyncInfo(on_wait=[w], on_update=[])
                        new.append(ev)
                    inst.sync_info = _br.SyncInfo(
                        on_wait=[waits[-1]], on_update=list(si.on_update)
                    )
                new.append(inst)
            ordered[bb_name] = new
        return _orig_lower(self, ordered)

    tile.TileContext._lower_ordered_insts = _split_lower
    tile.TileContext._drain_split_patch = True


def _install_ntff_hook():
    """Register the axon NTFF profile hook missing from this image's antenv,
    so run_bass_kernel_spmd(trace=True) can report HW exec time."""
    if "antenv.axon_hooks" in sys.modules:
        return
    try:
        import antenv
        from trn_agent_boot.trn_boot import _ntff_profile_via_ctypes

        hook = _ntff_profile_via_ctypes("/opt/axon/libaxon_pjrt.so")
        mod = types.ModuleType("antenv.axon_hooks")
        mod.get_axon_ntff_profile_hook = lambda: hook
        mod.set_axon_ntff_profile_hook = lambda h: None
        sys.modules["antenv.axon_hooks"] = mod
        antenv.axon_hooks = mod
    except Exception:
        pass  # tracing degrades; execution still works


# largest fp32 below 1.0: spike == Relu(Sign(v' - B0)) exactly, for every
# fp32 v' (B0 is the largest float < 1, v'-B0 is exact by Sterbenz on
# [0.5,2) and sign-exact elsewhere; the only zero case v'==B0 correctly
# yields no spike)
B0 = float(np.float32(1.0) - np.float32(2.0 ** -24))


def build_nc():
    """One SPMD NeuronCore program; all 8 cores run it on their own shard."""
    _patch_tile_drain()
    nc = bass.Bass()
    # register -B0 as a per-partition constant for the Sign activation bias
    # (the barrier is required: without it the race detector flags the
    # memset racing the first Sign's bias read)
    _cb = nc.alloc_sbuf_tensor("const-float32-negb0", [128, 1], F32)
    nc.gpsimd.memset(_cb.ap(), -B0)
    nc.const_aps.aps[(F32, -B0)] = _cb.ap()
    nc.all_engine_barrier()
    xT = nc.dram_tensor("xT", [T, D, S], F32, kind="ExternalInput")
    wf = nc.dram_tensor("wf", [P, 2 * DCH * 500], BF16, kind="ExternalInput")
    y2 = nc.dram_tensor("y2", [T, NG, P, 2 * C], F16, kind="ExternalOutput")
    yt = nc.dram_tensor("yt", [P, C], F16, kind="ExternalOutput")

    with tile.TileContext(nc) as tc:
        with (
            tc.tile_pool(name="wpool", bufs=1) as wpool,
            tc.tile_pool(name="vpool", bufs=1) as vpool,
            tc.tile_pool(name="xpool", bufs=3) as xpool,
            tc.tile_pool(name="spool", bufs=2) as spool,
            tc.tile_pool(name="opool", bufs=4) as opool,
            tc.tile_pool(name="ppool", bufs=2, space="PSUM") as ppool,
        ):
            def x3(xt):
                return xt[:, :].rearrange("p (d s) -> p d s", d=DCH)

            def sp3(sp):
                return sp[:, :].rearrange("p (d s) -> p d s", d=DCH)

            def load_x(t, xt, parts, eng):
                """DMA dram xT[t] ranges into the d-major flat tile; one
                descriptor per (partition, d) = (s1-s0)*4 contiguous bytes."""
                for s0, sn in parts:
                    eng.dma_start(
                        out=x3(xt)[:, :, s0 : s0 + sn],
                        in_=xT[t, :, s0 : s0 + sn].rearrange(
                            "(d p) s -> p d s", d=DCH
                        ),
                    )

            # startup: W solo on the Sync HWDGE ring (one 8KB descriptor per
            # partition) while ALL of x0 rides the GpSimd software-DGE queue
            # (r0 first) -- two descriptor feeds running in parallel so the
            # first matmul's inputs (W + r0 spikes) land together.
            wt = wpool.tile([P, 2 * DCH * 500], BF16, tag="w", name="w")
            nc.sync.dma_start(out=wt[:, :], in_=wf[:, :])
            x0 = xpool.tile([P, DCH * S], F32, tag="x", name="x0")
            load_x(0, x0, [(0, 256)], nc.gpsimd)
            load_x(0, x0, [(256, 512), (768, 800)], nc.gpsimd)

            def wslice(ci, d):
                return wt[:, (ci * DCH + d) * 500 : (ci * DCH + d + 1) * 500]

            def lif_range(t, xcur, xprev, v, sp, tp, ri):
                """Emit charge (+ previous step's reset) and threshold for
                sample range ri of timestep t."""
                s0, sn = RANGES[ri]
                mn = min(sn, SMAIN - s0)     # main part (to sp)
                xq = x3(xcur)[:, :, s0 : s0 + sn]
                if t == 0:
                    # v' = 0.5*x (exact; matches v + (x-v)/2 with v=0)
                    if CHARGE0_ENG[ri] == "S":
                        nc.scalar.activation(
                            out=xq, in_=xq, func=ACTF.Copy, scale=0.5
                        )
                    else:
                        nc.vector.tensor_scalar(
                            out=xq, in0=xq, scalar1=0.5, scalar2=None,
                            op0=ALU.mult,
                        )
                else:
                    # reset of the previous step is interleaved here, right
                    # before this range's charge: v = (v' < 1) * v' (exact)
                    xp = x3(xprev)[:, :, s0 : s0 + sn]
                    vq = x3(v)[:, :, s0 : s0 + sn]
                    nc.vector.scalar_tensor_tensor(
                        out=vq, in0=xp, scalar=1.0,
                        in1=xp, op0=ALU.is_lt, op1=ALU.mult,
                    )
                    # h = (x*1 - v), then v' = (h * 0.5) + v -- exact
                    # replication of the reference rounding order
                    nc.vector.scalar_tensor_tensor(
                        out=xq, in0=xq, scalar=1.0, in1=vq,
                        op0=ALU.mult, op1=ALU.subtract,
                    )
                    nc.vector.scalar_tensor_tensor(
                        out=xq, in0=xq, scalar=0.5, in1=vq,
                        op0=ALU.mult, op1=ALU.add,
                    )
                # threshold: spike = v' >= 1
                xm = x3(xcur)[:, :, s0 : s0 + mn]
                sq = sp3(sp)[:, :, s0 : s0 + mn]
                if THRESH_ENG[ri] == "S":
                    nc.scalar.activation(
                        out=sq, in_=xm, func=ACTF.Sign, bias=-B0
                    )
                    nc.scalar.activation(out=sq, in_=sq, func=ACTF.Relu)
                else:
                    nc.vector.tensor_scalar(
                        out=sq, in0=xm, scalar1=1.0, scalar2=None,
                        op0=ALU.is_ge,
                    )
                if s0 + sn > SMAIN:
                    # 32-sample tail -> packed cross-t tile (col 32*t + u)
                    xr = x3(xcur)[:, :, SMAIN:S]
                    tq = tp[:, :].rearrange("p (d u) -> p d u", d=DCH)[
                        :, :, STAIL * t : STAIL * (t + 1)
                    ]
                    nc.vector.tensor_scalar(
                        out=tq, in0=xr, scalar1=1.0, scalar2=None,
                        op0=ALU.is_ge,
                    )

            # packed tail spikes: col = d*128 + 32*t + tail-sample
            tp = vpool.tile([P, DCH * P], BF16, tag="tp", name="tp")
            v = vpool.tile([P, DCH * S], F32, tag="v", name="v")

            def emit_tail_group():
                # packed tail: one 128-row chunk covering the 32-row tails
                # of all 4 timesteps (psum partition = 32*t + tail sample);
                # emitted before t3's last paired group so its copy/DMA
                # overlap that group's matmuls instead of extending the end
                ps = ppool.tile([P, 4, 512], F32, tag="ps")
                ot = opool.tile([P, 2 * C], F16, tag="out")
                for ci, (c0, cn) in enumerate(CHALF):
                    for d in range(DCH):
                        nc.tensor.matmul(
                            ps[:P, ci, :cn],
                            tp[:, d * P : (d + 1) * P],
                            wslice(ci, d),
                            start=(d == 0),
                            stop=(d == DCH - 1),
                        )
                nc.scalar.copy(out=ot[:, :C], in_=ps[:, :2, :500])
                nc.sync.dma_start(out=yt[:, :], in_=ot[:, :C])

            xcur = None
            xprev = None
            xnext = x0
            for t in range(T):
                xcur, xnext = xnext, None
                sp = spool.tile([P, DCH * SMAIN], BF16, tag="sp", name=f"sp{t}")
                for ri in range(len(RANGES)):
                    lif_range(t, xcur, xprev, v, sp, tp, ri)
                xprev = xcur
                if t < T - 1:
                    xnext = xpool.tile(
                        [P, DCH * S], F32, tag="x", name=f"x{t+1}"
                    )
                if t == 0:
                    # x1's startup range rides the (otherwise y-only) Sync
                    # ring so t1's first spikes are ready before the t0->t1
                    # boundary even though the SWDGE queue is busy with x0
                    load_x(1, xnext, [(0, 512)], nc.sync)

                for g, chunks in enumerate(GROUPS):
                    if g == 0 and t < T - 1:
                        # prefetch x(t+1) on the software-DGE ring; its
                        # descriptor feed is independent of the y-store ring
                        parts = (
                            [(512, 1056)] if t == 0 else [(0, 512), (512, 1056)]
                        )
                        load_x(t + 1, xnext, parts, nc.gpsimd)
                    if g == 4 and t == T - 1:
                        emit_tail_group()
                    # 2 sample chunks x 2 C-halves per 4-bank PSUM group;
                    # ppool bufs=2 ping-pongs groups through all 8 banks.
                    ps = ppool.tile([P, 4, 512], F32, tag="ps")
                    ot = opool.tile([P, 2 * C], F16, tag="out")
                    for j, k in enumerate(chunks):
                        for ci, (c0, cn) in enumerate(CHALF):
                            for d in range(DCH):
                                s0 = d * SMAIN + P * k
                                nc.tensor.matmul(
                                    ps[:P, 2 * j + ci, :cn],
                                    sp[:, s0 : s0 + P],
                                    wslice(ci, d),
                                    start=(d == 0),
                                    stop=(d == DCH - 1),
                                )
                    if t == T - 1 and g == NG - 1:
                        # final group: copy/store in halves so the last DMA
                        # starts ~1us earlier and the tail drain is shorter
                        nc.scalar.copy(out=ot[:, :C], in_=ps[:, :2, :500])
                        nc.sync.dma_start(out=y2[t, g, :, 0:C], in_=ot[:, :C])
                        nc.scalar.copy(out=ot[:, C : 2 * C], in_=ps[:, 2:4, :500])
                        nc.sync.dma_start(
                            out=y2[t, g, :, C : 2 * C], in_=ot[:, C : 2 * C]
                        )
                    else:
                        nc.scalar.copy(
                            out=ot[:, : 4 * 500], in_=ps[:, :4, :500]
                        )
                        nc.sync.dma_start(out=y2[t, g], in_=ot[:, : 2 * C])

    return nc


_NC_CACHE = {}


def _get_nc():
    if "nc" not in _NC_CACHE:
        _NC_CACHE["nc"] = build_nc()
    return _NC_CACHE["nc"]


def _make_in_maps(x, W):
    # wf[p, ci, d, c] = W[ci*500+c, d*128+p] in bf16 -- the exact SBUF image
    wfi = np.ascontiguousarray(
        W.reshape(2, 500, DCH, P).transpose(3, 0, 2, 1).reshape(P, 2 * DCH * 500)
    ).astype(ml_dtypes.bfloat16)
    in_maps = []
    for c in range(NCORES):
        xc = x[:, c * BL : (c + 1) * BL].reshape(T, S, D)
        in_maps.append(
            {"xT": np.ascontiguousarray(xc.transpose(0, 2, 1)), "wf": wfi}
        )
    return in_maps


def kernel(x, W, b):
    from concourse.bass_utils import run_bass_kernel_spmd

    _install_ntff_hook()
    x = np.asarray(x, dtype=np.float32)
    W = np.asarray(W, dtype=np.float32)
    b = np.asarray(b, dtype=np.float32)

    nc = _get_nc()
    in_maps = _make_in_maps(x, W)
    res = run_bass_kernel_spmd(nc, in_maps, list(range(NCORES)))
    parts = []
    for c in range(NCORES):
        y2 = res.results[c]["y2"].astype(np.float32)  # [T, 6, P, 2000]
        ytl = res.results[c]["yt"].astype(np.float32)  # [P, C]
        ym = (
            y2.reshape(T, NG, P, 2, 2, 500)
            .transpose(0, 1, 3, 2, 4, 5)
            .reshape(T, SMAIN, C)
        )
        yc = np.concatenate([ym, ytl.reshape(T, STAIL, C)], axis=1)
        parts.append(yc.reshape(T, BL, N, C))
    y = np.concatenate(parts, axis=1)
    if np.any(b):
        y = y + b[None, None, None, :]
    return np.ascontiguousarray(y, dtype=np.float32)


# revision 7
# speedup vs baseline: 1.0038x; 1.0038x over previous
"""Trainium2 Bass kernel for nn_Decoder_10110353014984.

Computation (see reference): hard-reset LIF over T=4 steps followed by a
linear head:
    v' = v + (x_t - v)/2 ; spike = (v' >= 1) ; v = (1-spike) * v'
    y  = einsum('tbnd,cd->tbnc', spikes, W) + b

The LIF replicates the reference's exact fp32 rounding order:
    h = (x*1 - v) ; v' = h*0.5 + v ; spike = v' >= 1 ; v = (v' < 1) * v'
(x*1 and h*0.5 are exact, so the rounding sequence matches v + (x-v)/2).
Exactness matters: a single spike flip changes one output row by a full
W column and would blow the error budget.

Sharding: data-parallel over batch B=64 -> 8 per NeuronCore.

Data movement: the host ships x as three per-timestep SBUF-image slabs
(xa/xb/xc covering samples [0,256)/[256,768)/[768,1568) in the d-major
tile layout) so every DMA descriptor is a 4-13KB per-partition
contiguous run, and W as the exact flat SBUF image wf[128, 4000] (bf16,
one 8KB descriptor per partition).  x rides the GpSimd software-DGE
queue while W + y-stores ride the Sync HWDGE ring - two descriptor
feeds in parallel.  y goes out as [T, 6, P, 2000] (4KB per-partition
descriptors) plus a packed 32-row tail tensor; the host untangles it.
x(t+1) is prefetched at the start of timestep t; x1's first slab rides
the Sync ring so the t0->t1 boundary is covered while the SWDGE queue
drains x0.

LIF charge/reset ranges are 1:1 with the DMA slabs so the tile
scheduler's DMA-completion estimates order the DVE chain correctly.
Thresholds are split DVE/Scalar by sample range to balance both engines
just under the PE's ~88us matmul stream; PSUM->SBUF(fp16) copies run on
Scalar, 2 sample-chunks x 2 C-halves per instruction.  A short burst of
dummy warm-up matmuls during the DMA fill ramps the PE out of its low
p-state (cold PE streams at 1.2GHz, warm at 2.4GHz) before the real
stream begins.  The four 32-row tails (S = 12*128 + 32) are packed
across t into one full 128-row matmul chunk at the end.
"""

import sys
import types

sys.path.insert(0, "/opt/trn_rl_repo")

import numpy as np
import ml_dtypes

import concourse.bass as bass
import concourse.mybir as mybir
import concourse.tile as tile
from concourse.vector_clock import ScopedClock
import bass_rust as _br

T, B, N, D, C = 4, 64, 196, 512, 1000
NCORES = 8
BL = B // NCORES          # 8 batches per core
S = BL * N                # 1568 samples per timestep per core
P = 128                   # partition width
DCH = D // P              # 4 contraction tiles
CHALF = [(0, 500), (500, 500)]  # C split across two PSUM banks
GROUPS = [(0, 1), (2, 3), (4, 5), (6, 7), (8, 9), (10, 11)]
NG = len(GROUPS)
SMAIN = 12 * P            # 1536 samples in the paired groups
STAIL = S - SMAIN         # 32 tail samples per timestep

# x DMA slabs == LIF charge ranges (kick-aligned so the scheduler's DMA
# estimates order the DVE chain correctly)
SLABS = [(0, 256), (256, 512), (768, 800)]
# threshold pieces (s0, sn, engine); 'V' = DVE is_ge (1 pass),
# 'S' = Scalar Sign+Relu (2 passes); last piece carries the 32-wide tail
THRESH = [(0, 256, "V"), (256, 512, "S"), (768, 512, "S"), (1280, 288, "V")]
WARMUP_MM = 20            # dummy matmuls to ramp the PE p-state

F32 = mybir.dt.float32
F16 = mybir.dt.float16
BF16 = mybir.dt.bfloat16
ALU = mybir.AluOpType
ACTF = mybir.ActivationFunctionType


def _patch_tile_drain():
    """This walrus build allows at most one sync wait per TPB_CTRL (Drain)
    instruction; Tile's tail drain carries one wait per active processor.
    Split it into a chain of single-wait drains (same-engine program order
    makes the conjunction equivalent)."""
    if getattr(tile.TileContext, "_drain_split_patch", False):
        return

    def _drain_and_barrier(self, tick_clock, wait_clock):
        drain_inst = self.nc.sync.drain()
        wait_clock.add_sem_waits(
            drain_inst.ins, ScopedClock({None: tick_clock.global_clock})
        )
        waits = (
            list(drain_inst.ins.sync_info.on_wait)
            if drain_inst.ins.has_wait()
            else []
        )
        if len(waits) > 1:
            drain_inst.ins.sync_info.on_wait = waits[:1]
            for i in range(1, len(waits)):
                d2 = self.nc.sync.drain()
                d2.ins.sync_info = _br.SyncInfo(on_wait=waits[i : i + 1], on_update=[])
        self.nc.all_engine_barrier()
        assert self.sems is not None
        popped = self.nc._tile_sem_poison_stack.pop()
        assert popped is self._sem_poison
        self.nc.clear_and_free_semaphores(list(self.sems.allocated().values()))
        self.nc.all_engine_barrier()

    tile.TileContext._drain_and_barrier = _drain_and_barrier

    # Same limit applies to every instruction class (Matmult, DMACopy, ...).
    # Before committing the scheduled instruction stream, shed all but one
    # wait per instruction onto standalone same-engine InstEventSemaphore
    # carriers placed immediately before it (engine program order preserves
    # the conjunction).
    _orig_lower = tile.TileContext._lower_ordered_insts

    def _split_lower(self, ordered):
        for bb_name, insts in ordered.items():
            new = []
            for inst in insts:
                si = inst.sync_info
                if si is not None and len(si.on_wait) > 1:
                    waits = list(si.on_wait)
                    for w in waits[:-1]:
                        ev = mybir.InstEventSemaphore(
                            name=self.nc.get_next_instruction_name(), ins=[], outs=[]
                        )
                        ev.engine = inst.engine
                        ev.sync_info = _br.SyncInfo(on_wait=[w], on_update=[])
                        new.append(ev)
                    inst.sync_info = _br.SyncInfo(
                        on_wait=[waits[-1]], on_update=list(si.on_update)
                    )
                new.append(inst)
            ordered[bb_name] = new
        return _orig_lower(self, ordered)

    tile.TileContext._lower_ordered_insts = _split_lower
    tile.TileContext._drain_split_patch = True


def _install_ntff_hook():
    """Register the axon NTFF profile hook missing from this image's antenv,
    so run_bass_kernel_spmd(trace=True) can report HW exec time."""
    if "antenv.axon_hooks" in sys.modules:
        return
    try:
        import antenv
        from trn_agent_boot.trn_boot import _ntff_profile_via_ctypes

        hook = _ntff_profile_via_ctypes("/opt/axon/libaxon_pjrt.so")
        mod = types.ModuleType("antenv.axon_hooks")
        mod.get_axon_ntff_profile_hook = lambda: hook
        mod.set_axon_ntff_profile_hook = lambda h: None
        sys.modules["antenv.axon_hooks"] = mod
        antenv.axon_hooks = mod
    except Exception:
        pass  # tracing degrades; execution still works


# largest fp32 below 1.0: spike == Relu(Sign(v' - B0)) exactly, for every
# fp32 v' (B0 is the largest float < 1, v'-B0 is exact by Sterbenz on
# [0.5,2) and sign-exact elsewhere; the only zero case v'==B0 correctly
# yields no spike)
B0 = float(np.float32(1.0) - np.float32(2.0 ** -24))


def build_nc():
    """One SPMD NeuronCore program; all 8 cores run it on their own shard."""
    _patch_tile_drain()
    nc = bass.Bass()
    # register -B0 as a per-partition constant for the Sign activation bias
    # plus a zeroed scratch for the PE warm-up matmuls (the barrier orders
    # the memsets before their cross-engine readers)
    _cb = nc.alloc_sbuf_tensor("const-float32-negb0", [128, 1], F32)
    nc.gpsimd.memset(_cb.ap(), -B0)
    nc.const_aps.aps[(F32, -B0)] = _cb.ap()
    _warm = nc.alloc_sbuf_tensor("warm-scratch", [128, 128], BF16)
    nc.gpsimd.memset(_warm[:, :], 0.0)
    nc.all_engine_barrier()
    # PE p-state warm-up: short dummy matmuls that run during the DMA fill
    # (no waits), pulling the PE to full clock before the real stream
    with nc.psum_tensor("warm-psum", [P, P], F32) as wp:
        for _ in range(WARMUP_MM):
            nc.tensor.matmul(
                wp[:, :], _warm[:, :], _warm[:, :], start=True, stop=True
            )

    xT = {}
    for name, (s0, sn) in zip("abc", SLABS):
        xT[s0] = nc.dram_tensor(
            f"x{name}", [T, P, DCH * sn], F32, kind="ExternalInput"
        )
    wf = nc.dram_tensor("wf", [P, 2 * DCH * 500], BF16, kind="ExternalInput")
    y2 = nc.dram_tensor("y2", [T, NG, P, 2 * C], F16, kind="ExternalOutput")
    yt = nc.dram_tensor("yt", [P, C], F16, kind="ExternalOutput")

    with tile.TileContext(nc) as tc:
        with (
            tc.tile_pool(name="wpool", bufs=1) as wpool,
            tc.tile_pool(name="vpool", bufs=1) as vpool,
            tc.tile_pool(name="xpool", bufs=3) as xpool,
            tc.tile_pool(name="spool", bufs=2) as spool,
            tc.tile_pool(name="opool", bufs=4) as opool,
            tc.tile_pool(name="ppool", bufs=2, space="PSUM") as ppool,
        ):
            def x3(xt):
                return xt[:, :].rearrange("p (d s) -> p d s", d=DCH)

            def sp3(sp):
                return sp[:, :].rearrange("p (d s) -> p d s", d=DCH)

            def load_x(t, xt, slabs, eng):
                """DMA one or more x slabs into the d-major flat tile; the
                slab dram layout IS the SBUF image, so each descriptor is a
                (4*sn*4)B contiguous per-partition run."""
                for s0, sn in slabs:
                    eng.dma_start(
                        out=x3(xt)[:, :, s0 : s0 + sn],
                        in_=xT[s0][t].rearrange("p (d s) -> p d s", d=DCH),
                    )

            # startup: W solo on the Sync HWDGE ring while all of x0 rides
            # the GpSimd software-DGE queue (slab a first) -- two
            # descriptor feeds running in parallel.
            wt = wpool.tile([P, 2 * DCH * 500], BF16, tag="w", name="w")
            nc.sync.dma_start(out=wt[:, :], in_=wf[:, :])
            x0 = xpool.tile([P, DCH * S], F32, tag="x", name="x0")
            load_x(0, x0, SLABS, nc.gpsimd)

            def wslice(ci, d):
                return wt[:, (ci * DCH + d) * 500 : (ci * DCH + d + 1) * 500]

            def lif_slab(t, xcur, xprev, v, ri):
                """Charge (+ previous step's reset) for slab ri of step t."""
                s0, sn = SLABS[ri]
                xq = x3(xcur)[:, :, s0 : s0 + sn]
                if t == 0:
                    # v' = 0.5*x (exact; matches v + (x-v)/2 with v=0)
                    nc.vector.tensor_scalar(
                        out=xq, in0=xq, scalar1=0.5, scalar2=None,
                        op0=ALU.mult,
                    )
                    return
                # reset of the previous step, right before this slab's
                # charge: v = (v' < 1) * v' (exact hard reset)
                xp = x3(xprev)[:, :, s0 : s0 + sn]
                vq = x3(v)[:, :, s0 : s0 + sn]
                nc.vector.scalar_tensor_tensor(
                    out=vq, in0=xp, scalar=1.0,
                    in1=xp, op0=ALU.is_lt, op1=ALU.mult,
                )
                # h = (x*1 - v), then v' = (h * 0.5) + v -- exact
                # replication of the reference rounding order
                nc.vector.scalar_tensor_tensor(
                    out=xq, in0=xq, scalar=1.0, in1=vq,
                    op0=ALU.mult, op1=ALU.subtract,
                )
                nc.vector.scalar_tensor_tensor(
                    out=xq, in0=xq, scalar=0.5, in1=vq,
                    op0=ALU.mult, op1=ALU.add,
                )

            def thresh_piece(t, xcur, sp, tp, pi):
                """spike = v' >= 1 for threshold piece pi."""
                s0, sn, eng = THRESH[pi]
                mn = min(sn, SMAIN - s0)
                xm = x3(xcur)[:, :, s0 : s0 + mn]
                sq = sp3(sp)[:, :, s0 : s0 + mn]
                if eng == "S":
                    nc.scalar.activation(
                        out=sq, in_=xm, func=ACTF.Sign, bias=-B0
                    )
                    nc.scalar.activation(out=sq, in_=sq, func=ACTF.Relu)
                else:
                    nc.vector.tensor_scalar(
                        out=sq, in0=xm, scalar1=1.0, scalar2=None,
                        op0=ALU.is_ge,
                    )
                if s0 + sn > SMAIN:
                    # 32-sample tail -> packed cross-t tile (col 32*t + u)
                    xr = x3(xcur)[:, :, SMAIN:S]
                    tq = tp[:, :].rearrange("p (d u) -> p d u", d=DCH)[
                        :, :, STAIL * t : STAIL * (t + 1)
                    ]
                    nc.vector.tensor_scalar(
                        out=tq, in0=xr, scalar1=1.0, scalar2=None,
                        op0=ALU.is_ge,
                    )

            # packed tail spikes: col = d*128 + 32*t + tail-sample
            tp = vpool.tile([P, DCH * P], BF16, tag="tp", name="tp")
            v = vpool.tile([P, DCH * S], F32, tag="v", name="v")

            def emit_tail_group():
                # packed tail: one 128-row chunk covering the 32-row tails
                # of all 4 timesteps (psum partition = 32*t + tail sample);
                # emitted before t3's last paired group so its copy/DMA
                # overlap that group's matmuls instead of extending the end
                ps = ppool.tile([P, 4, 512], F32, tag="ps")
                ot = opool.tile([P, 2 * C], F16, tag="out")
                for ci, (c0, cn) in enumerate(CHALF):
                    for d in range(DCH):
                        nc.tensor.matmul(
                            ps[:P, ci, :cn],
                            tp[:, d * P : (d + 1) * P],
                            wslice(ci, d),
                            start=(d == 0),
                            stop=(d == DCH - 1),
                        )
                nc.scalar.copy(out=ot[:, :C], in_=ps[:, :2, :500])
                nc.sync.dma_start(out=yt[:, :], in_=ot[:, :C])

            xcur = None
            xprev = None
            xnext = x0
            for t in range(T):
                xcur, xnext = xnext, None
                sp = spool.tile([P, DCH * SMAIN], BF16, tag="sp", name=f"sp{t}")
                # charge slab-by-slab, each followed by its threshold
                # pieces, so spikes stream out in consumption order
                for ri in range(len(SLABS)):
                    lif_slab(t, xcur, xprev, v, ri)
                    r0, rn = SLABS[ri]
                    for pi, (s0, sn, eng) in enumerate(THRESH):
                        if r0 <= s0 < r0 + rn:
                            thresh_piece(t, xcur, sp, tp, pi)
                xprev = xcur
                if t < T - 1:
                    xnext = xpool.tile(
                        [P, DCH * S], F32, tag="x", name=f"x{t+1}"
                    )
                if t == 0:
                    # x1's first slab rides the (otherwise y-only) Sync
                    # ring so t1's first spikes are ready at the boundary
                    # even though the SWDGE queue is still busy with x0
                    load_x(1, xnext, SLABS[:1], nc.sync)

                for g, chunks in enumerate(GROUPS):
                    if g == 0 and t < T - 1:
                        # prefetch x(t+1) on the software-DGE ring; its
                        # descriptor feed is independent of the y-store ring
                        parts = SLABS[1:] if t == 0 else SLABS
                        load_x(t + 1, xnext, parts, nc.gpsimd)
                    if g == 4 and t == T - 1:
                        emit_tail_group()
                    # 2 sample chunks x 2 C-halves per 4-bank PSUM group;
                    # ppool bufs=2 ping-pongs groups through all 8 banks.
                    ps = ppool.tile([P, 4, 512], F32, tag="ps")
                    ot = opool.tile([P, 2 * C], F16, tag="out")
                    for j, k in enumerate(chunks):
                        for ci, (c0, cn) in enumerate(CHALF):
                            for d in range(DCH):
                                s0 = d * SMAIN + P * k
                                nc.tensor.matmul(
                                    ps[:P, 2 * j + ci, :cn],
                                    sp[:, s0 : s0 + P],
                                    wslice(ci, d),
                                    start=(d == 0),
                                    stop=(d == DCH - 1),
                                )
                    if t == T - 1 and g == NG - 1:
                        # final group: copy/store in halves so the last DMA
                        # starts ~1us earlier and the tail drain is shorter
                        nc.scalar.copy(out=ot[:, :C], in_=ps[:, :2, :500])
                        nc.sync.dma_start(out=y2[t, g, :, 0:C], in_=ot[:, :C])
                        nc.scalar.copy(out=ot[:, C : 2 * C], in_=ps[:, 2:4, :500])
                        nc.sync.dma_start(
                            out=y2[t, g, :, C : 2 * C], in_=ot[:, C : 2 * C]
                        )
                    else:
                        nc.scalar.copy(
                            out=ot[:, : 4 * 500], in_=ps[:, :4, :500]
                        )
                        nc.sync.dma_start(out=y2[t, g], in_=ot[:, : 2 * C])

    return nc


_NC_CACHE = {}


def _get_nc():
    if "nc" not in _NC_CACHE:
        _NC_CACHE["nc"] = build_nc()
    return _NC_CACHE["nc"]


def _make_in_maps(x, W):
    # wf[p, ci, d, c] = W[ci*500+c, d*128+p] in bf16 -- the exact SBUF image
    wfi = np.ascontiguousarray(
        W.reshape(2, 500, DCH, P).transpose(3, 0, 2, 1).reshape(P, 2 * DCH * 500)
    ).astype(ml_dtypes.bfloat16)
    in_maps = []
    for c in range(NCORES):
        xc = x[:, c * BL : (c + 1) * BL].reshape(T, S, D)
        # d-major SBUF image per slab: x_img[t, p, d, s] = xc[t, s, d*128+p]
        xTt = xc.transpose(0, 2, 1).reshape(T, DCH, P, S)  # [t, d, p, s]
        m = {"wf": wfi}
        for name, (s0, sn) in zip("abc", SLABS):
            m[f"x{name}"] = np.ascontiguousarray(
                xTt[:, :, :, s0 : s0 + sn].transpose(0, 2, 1, 3)
            ).reshape(T, P, DCH * sn)
        in_maps.append(m)
    return in_maps


def kernel(x, W, b):
    from concourse.bass_utils import run_bass_kernel_spmd

    _install_ntff_hook()
    x = np.asarray(x, dtype=np.float32)
    W = np.asarray(W, dtype=np.float32)
    b = np.asarray(b, dtype=np.float32)

    nc = _get_nc()
    in_maps = _make_in_maps(x, W)
    res = run_bass_kernel_spmd(nc, in_maps, list(range(NCORES)))
    parts = []
    for c in range(NCORES):
        y2 = res.results[c]["y2"].astype(np.float32)  # [T, 6, P, 2000]
        ytl = res.results[c]["yt"].astype(np.float32)  # [P, C]
        ym = (
            y2.reshape(T, NG, P, 2, 2, 500)
            .transpose(0, 1, 3, 2, 4, 5)
            .reshape(T, SMAIN, C)
        )
        yc = np.concatenate([ym, ytl.reshape(T, STAIL, C)], axis=1)
        parts.append(yc.reshape(T, BL, N, C))
    y = np.concatenate(parts, axis=1)
    if np.any(b):
        y = y + b[None, None, None, :]
    return np.ascontiguousarray(y, dtype=np.float32)
